# revision 35
# baseline (speedup 1.0000x reference)
"""Trainium2 Bass kernel for CalcSpixelFeats (superpixel feature aggregation).

Strategy ("sorted-segment matmul", bf16 edition):
  - 8 NeuronCores, each handles half an image (4 images x 2 pixel-halves).
  - Host-side sharding/layout: stable-sort each core's pixels by their base
    superpixel id. Every 128-pixel tile then touches at most 3 id-groups,
    split at two boundary offsets s1 <= s2.  The host packs ONE interleaved
    bf16 DRAM tensor with 43 columns per tile, chunk-transposed so the tile
    index is the innermost (stride-1) dim:
      [F(32ch) | W(9) | maskA | maskAB]
    where maskA[p] = (p < s1), maskAB[p] = (p < s2) are precomputed 0/1.
  - Device, per 64-tile chunk: ONE DMA (fewer DMAs measured strictly faster
    on HW: each costs ~1us in dispatch + completion semaphores); DVE builds
    the 27-row stationary operand [W*maskA | W*maskAB | W] with two
    broadcast-multiplies and a copy (all operands end in a packed dim, so
    the 16-bit 2x DVE mode applies); per tile ONE bf16 matmul
    out[27, 32] = wcat[128, 27]^T @ F[128, 32], rotating over 4 PE column
    quadrants x 16 slots x 4 PSUM banks (exactly 256 tiles).  Each full
    bank is cast-copied by the scalar engine to fp16 staging and DMA'd out.
  - Weight sums are computed HOST-side over the same bf16 weights (exact),
    which drops the ones-column from the stream and makes PSUM banks pack
    exactly.
  - Host-side unshard: prefix-difference the 3 planes into per-group partial
    sums, scatter-add into per-image bins, apply the 3x3 neighbor offsets
    with validity, divide by weight sums (fp64).

bf16 inputs / fp32 PSUM accumulate / fp16 partial-sum output gives rel err
1.9e-3 on the fixed harness input, far inside the 2e-2 gate (fp8 variants
were checked exactly on the real input: 2.1e-2..3.0e-2 -> rejected).
"""
import sys

sys.path.insert(0, "/opt/trn_rl_repo")

import numpy as np
import ml_dtypes

B, C, H, W = 4, 32, 256, 256
NW = NH = 16
K = NW * NH
PIX = H * W // 2          # pixels per core (half image)
TILE = 128
T = PIX // TILE           # 256 tiles per core
NCOLS = C + 1             # 33: channels + host-side wsum (merge layout)
MCOLS = 27                # 3 planes x 9
TCOLS = C + 9 + 2         # 43: F32 | W9 | maskA | maskAB
BPB = 16                  # 32-col block slots per PSUM bank (512 fp32 exact)
NBANKS = 4                # col-packed: 4 groups x 16 slots = 64 tiles per bank
STG = NBANKS * BPB * C    # 2048 output columns
CHUNK = 64                # tiles per DMA chunk (bench env compat)
# Uniform chunks measured fastest on HW: per-DMA dispatch + completion
# semaphore costs (~1us each) outweigh the model's predicted pipeline wins
# from small head/tail chunks.
CHUNKS = [CHUNK] * (T // CHUNK)
N_CORES = 8

_compiled = None


class _CompiledKernel:
    """Compile a finalized Bass module once; run SPMD on 8 cores via PJRT."""

    def __init__(self, nc, n_cores):
        import jax
        import concourse.mybir as mybir
        from concourse.bass2jax import (
            _bass_exec_p, partition_id_tensor, install_neuronx_cc_hook)
        from jax.sharding import Mesh, PartitionSpec
        from jax.experimental.shard_map import shard_map

        install_neuronx_cc_hook()
        if not nc.is_finalized():
            nc.finalize()
        self.nc = nc
        self.n_cores = n_cores
        self._jax = jax
        partition_name = (nc.partition_id_tensor.name
                          if nc.partition_id_tensor else None)
        in_names, out_names, out_avals = [], [], []
        for alloc in nc.m.functions[0].allocations:
            if not isinstance(alloc, mybir.MemoryLocationSet):
                continue
            name = alloc.memorylocations[0].name
            if alloc.kind == "ExternalInput":
                if name != partition_name:
                    in_names.append(name)
            elif alloc.kind == "ExternalOutput":
                out_names.append(name)
                out_avals.append(jax.core.ShapedArray(
                    tuple(alloc.tensor_shape), mybir.dt.np(alloc.dtype)))
        self.in_names, self.out_names, self.out_avals = in_names, out_names, out_avals
        n_params, n_outs = len(in_names), len(out_avals)
        all_in_names = in_names + out_names
        if partition_name is not None:
            all_in_names.append(partition_name)

        def _body(*args):
            operands = list(args)
            if partition_name is not None:
                operands.append(partition_id_tensor())
            return tuple(_bass_exec_p.bind(
                *operands,
                out_avals=tuple(out_avals),
                in_names=tuple(all_in_names),
                out_names=tuple(out_names),
                lowering_input_output_aliases=(),
                sim_require_finite=True,
                sim_require_nnan=True,
                nc=nc,
            ))

        devices = jax.devices()[:n_cores]
        mesh = Mesh(np.asarray(devices), ("core",))
        self.fn = jax.jit(
            shard_map(_body, mesh=mesh,
                      in_specs=(PartitionSpec("core"),) * (n_params + n_outs),
                      out_specs=(PartitionSpec("core"),) * n_outs,
                      check_rep=False),
            keep_unused=True,
        )
        self._dev_args = None

    def set_inputs(self, in_maps):
        jax = self._jax
        concat_in = [
            np.concatenate([np.asarray(in_maps[c][name])
                            for c in range(self.n_cores)], axis=0)
            for name in self.in_names
        ]
        concat_zeros = [
            np.zeros((self.n_cores * a.shape[0], *a.shape[1:]), a.dtype)
            for a in self.out_avals
        ]
        self._dev_args = ([jax.device_put(a) for a in concat_in]
                          + [jax.device_put(z) for z in concat_zeros])

    def run_blocking(self):
        outs = self.fn(*self._dev_args)
        self._jax.block_until_ready(outs)
        return outs

    def get_results(self):
        outs = self.run_blocking()
        res = []
        for c in range(self.n_cores):
            d = {}
            for i, name in enumerate(self.out_names):
                per = np.asarray(outs[i]).reshape(
                    self.n_cores, *self.out_avals[i].shape)
                d[name] = per[c]
            res.append(d)
        return res


def _build_device(repeat=None):
    import contextlib
    import concourse.bacc as bacc
    import concourse.mybir as mybir
    from concourse import tile
    CompiledKernel = _CompiledKernel

    DT = mybir.dt.bfloat16
    DTO = mybir.dt.float16
    nc = bacc.Bacc("TRN2", target_bir_lowering=False, debug=False,
                   num_devices=N_CORES)
    IN_ = nc.dram_tensor("IN", [TILE, T * TCOLS], DT, kind="ExternalInput")
    OUTS = nc.dram_tensor("OUTS", [128, STG], DTO, kind="ExternalOutput")

    nchunks = T // CHUNK
    # (dram column offset, width) per PSUM bank
    bank_off = [(b * BPB * C, BPB * C) for b in range(NBANKS)]
    flush_tiles = {64 * (b + 1) - 1: b for b in range(NBANKS)}
    with tile.TileContext(nc) as tc:
        with (
            tc.tile_pool(name="fp", bufs=8) as fp,
            tc.tile_pool(name="st", bufs=1) as st,
            tc.tile_pool(name="ps", bufs=1, space="PSUM") as ps,
        ):
            loop_cm = (contextlib.nullcontext(0) if repeat is None
                       else tc.For_i(0, repeat))
            with loop_cm:
                stage_t = st.tile([128, STG], DTO, name="stage")
                stage = [stage_t[:, off:off + w] for off, w in bank_off]
                psums = [ps.tile([128, BPB * C], mybir.dt.float32,
                                 name=f"psbank{i}") for i in range(NBANKS)]
                t0 = 0
                for k, csz in enumerate(CHUNKS):
                    # chunk layout: [44, csz] per partition —
                    # rows 0:33 F+ones, 33:42 W9, 42 maskA, 43 maskAB.
                    # Keeping tiles as the innermost (stride-1) dim lets
                    # every DVE operand end in a packed dim (2x mode).
                    blk = fp.tile([TILE, TCOLS, csz], DT, name="blk")
                    nc.sync.dma_start(
                        out=blk[:],
                        in_=IN_[:, t0 * TCOLS:(t0 + csz) * TCOLS])
                    wcat = fp.tile([TILE, MCOLS, csz], DT, name="wcat")
                    w_v = blk[:, C:C + 9, :]
                    # rows 0:9 = W*maskA; 9:18 = W*maskAB; 18:27 = W
                    nc.vector.tensor_tensor(
                        wcat[:, 0:9, :], w_v,
                        blk[:, C + 9:C + 10, :].broadcast_to(
                            [TILE, 9, csz]),
                        op=mybir.AluOpType.mult,
                    )
                    nc.vector.tensor_tensor(
                        wcat[:, 9:18, :], w_v,
                        blk[:, C + 10:C + 11, :].broadcast_to(
                            [TILE, 9, csz]),
                        op=mybir.AluOpType.mult,
                    )
                    nc.vector.tensor_copy(wcat[:, 18:27, :], w_v)
                    for tt in range(csz):
                        t = t0 + tt
                        bank = t // 64
                        idx = t % 64
                        g = idx % 4
                        slot = idx // 4
                        nc.tensor.matmul(
                            psums[bank][32 * g:32 * g + MCOLS,
                                        slot * C:(slot + 1) * C],
                            wcat[:, :, tt],
                            blk[:, 0:C, tt],
                            start=True, stop=True, skip_group_check=True,
                            tile_position=(0, 32 * g),
                        )
                        if t in flush_tiles:
                            b = flush_tiles[t]
                            off, w = bank_off[b]
                            nc.scalar.copy(
                                out=stage[b],
                                in_=psums[b][:, 0:w],
                            )
                            nc.scalar.dma_start(
                                out=OUTS[:, off:off + w],
                                in_=stage_t[:, off:off + w])
                    t0 += csz
    return CompiledKernel(nc, N_CORES)


def _get_compiled():
    global _compiled
    if _compiled is None:
        _compiled = _build_device()
    return _compiled


def _prep_core(pf_half, am_half, idx_half):
    """pf_half: [C, PIX] f32, am_half: [9, PIX] f32, idx_half: [PIX] int.
    Returns (device input dict, (gA, gB, gC) merge metadata)."""
    order = np.argsort(idx_half, kind="stable")
    sid = idx_half[order].reshape(T, TILE)
    gA = sid[:, 0]
    neq = sid != gA[:, None]
    s1 = np.where(neq.any(1), neq.argmax(1), TILE).astype(np.int64)
    gB = sid[np.arange(T), np.minimum(s1, TILE - 1)]
    neq2 = (sid != gB[:, None]) & (np.arange(TILE)[None, :] >= s1[:, None])
    s2 = np.where(neq2.any(1), neq2.argmax(1), TILE).astype(np.int64)
    gC = sid[np.arange(T), np.minimum(s2, TILE - 1)]
    if (s2 < TILE).any():
        bad = np.nonzero(s2 < TILE)[0]
        for t in bad:
            assert (sid[t, s2[t]:] == gC[t]).all(), "tile spans >3 groups"

    pix = np.arange(TILE)
    # per-chunk layout [TCOLS, csz]: rows 0:32 F, 32:41 W, 41:43 masks
    Fs = pf_half[:, order].reshape(C, T, TILE)       # [C, T, P]
    Wso = am_half[:, order].reshape(9, T, TILE)      # [9, T, P]
    mA = (pix[:, None] < s1[None, :])                # [P, T]
    mAB = (pix[:, None] < s2[None, :])
    IN = np.empty((TILE, T * TCOLS), np.float32)
    t0 = 0
    for csz in CHUNKS:
        blk = IN[:, t0 * TCOLS:(t0 + csz) * TCOLS].reshape(TILE, TCOLS, csz)
        blk[:, :C, :] = Fs[:, t0:t0 + csz, :].transpose(2, 0, 1)
        blk[:, C:C + 9, :] = Wso[:, t0:t0 + csz, :].transpose(2, 0, 1)
        blk[:, C + 9, :] = mA[:, t0:t0 + csz]
        blk[:, C + 10, :] = mAB[:, t0:t0 + csz]
        t0 += csz
    # host-side weight sums over exactly the bf16 weights the device sees:
    # ws27[t, plane*9+j] = sum_p wcat[p, t, plane*9+j]
    w_bf = Wso.astype(ml_dtypes.bfloat16).astype(np.float64)  # [9, T, P]
    s_full = w_bf.sum(axis=2).T                               # [T, 9]
    s_a = np.einsum('jtp,pt->tj', w_bf, mA)
    s_ab = np.einsum('jtp,pt->tj', w_bf, mAB)
    ws27 = np.concatenate([s_a, s_ab, s_full], axis=1)        # [T, 27]
    inp = dict(IN=IN.astype(ml_dtypes.bfloat16))
    meta = (gA.astype(np.int64), gB.astype(np.int64), gC.astype(np.int64),
            ws27.astype(np.float32))
    return inp, meta


def _merge_core(outs, meta, bins):
    gA, gB, gC, ws27 = meta
    outs = outs.astype(np.float32)
    ts = np.arange(T)
    bk, idx = ts // 64, ts % 64
    g, sl = idx % 4, idx // 4
    cols = (bk[:, None] * BPB * C + sl[:, None] * C
            + np.arange(C)[None, :])
    rows = 32 * g[:, None] + np.arange(MCOLS)[None, :]
    blocks = outs[rows[:, :, None], cols[:, None, :]]   # [T, 27, 32]
    blocks = np.concatenate([blocks, ws27[:, :, None]], axis=2)  # [T, 27, 33]
    cA = blocks[:, 0:9, :]
    cB = blocks[:, 9:18, :] - cA
    cC = blocks[:, 18:27, :] - blocks[:, 9:18, :]
    np.add.at(bins, gA, cA)
    np.add.at(bins, gB, cB)
    np.add.at(bins, gC, cC)


def _finalize(bins_all):
    ks = np.arange(K)
    ix, iy = ks % NW, ks // NW
    fsum = np.zeros((B, C, K), np.float64)
    wsum = np.zeros((B, K), np.float64)
    j = 0
    for dy in (-1, 0, 1):
        for dx in (-1, 0, 1):
            tx, ty = ix + dx, iy + dy
            valid = (tx >= 0) & (tx < NW) & (ty >= 0) & (ty < NH)
            tgt = (ty * NW + tx)[valid]
            src = ks[valid]
            for b in range(B):
                np.add.at(fsum[b].T, tgt, bins_all[b, src, j, :C])
                np.add.at(wsum[b], tgt, bins_all[b, src, j, C])
            j += 1
    eps = 1e-16
    denom = np.where(wsum > eps, wsum, 1.0)
    out = np.where(wsum[:, None, :] > eps, fsum / denom[:, None, :], 0.0)
    return out.astype(np.float32)


def kernel(pixel_feats, assoc_map, index_map, nw_spixels, nh_spixels):
    assert int(nw_spixels) == NW and int(nh_spixels) == NH
    pixel_feats = np.asarray(pixel_feats, dtype=np.float32)
    assoc_map = np.asarray(assoc_map, dtype=np.float32)
    index_map = np.asarray(index_map)

    in_maps, metas = [], []
    for b in range(B):
        pf = pixel_feats[b].reshape(C, 2, PIX)
        am = assoc_map[b].reshape(9, 2, PIX)
        im = index_map[b].reshape(2, PIX)
        for h in range(2):
            inp, meta = _prep_core(pf[:, h], am[:, h], im[h].astype(np.int64))
            in_maps.append(inp)
            metas.append(meta)

    ck = _get_compiled()
    ck.set_inputs(in_maps)
    results = ck.get_results()

    bins_all = np.zeros((B, K, 9, NCOLS), np.float32)
    for core in range(N_CORES):
        _merge_core(results[core]["OUTS"], metas[core], bins_all[core // 2])
    return _finalize(bins_all)
